# revision 25
# baseline (speedup 1.0000x reference)
"""Trainium2 Bass kernel for the 2-layer dependency-relation GCN (8 cores).

Math per layer l, token i:
    out[i] = relu( W_self[l] @ x[i] + b_self[l]
                   + sum_{e: dep[e]==i} (W_rel[l, rel[e]]   @ x[gov[e]] + b_rel[l, rel[e]])
                   + sum_{e: gov[e]==i} (W_rel[l, R+rel[e]] @ x[dep[e]] + b_rel[l, R+rel[e]]) )
final:  y = h @ W_ff.T + b_ff

Relation-sharded message passing (5 of the 40 directed relations per core).
v4 schedule:
  * UNEVEN halves (5/3 sub-blocks): the trailing A2A + accumulation (the
    un-overlappable tail) covers only 3/8 of the tokens.
  * layer-1 message tiles are ordered source-half-pure-first per dest half,
    and gather chunks whose sources all lie in the first AllGather half read
    a SLICE of h_full -- so they fire right after AG-half-0 completes and
    overlap AG-half-1 on the wire.
  * gathers/scatters spread over 4 SWDGE queues; bulk constants ride the
    SWDGE mainline ring behind the gathers; only small latency-critical
    loads use the (single, shared) HWDGE engine.
  * no warm-up collective: it blocks the serial ncfw stream for ~18us right
    when the first real A2A is ready.
  * layer-1 accumulation in direct orientation + PE-transposed h2 feeding
    the fused FF GEMM.
Numerics: bf16 matmul inputs / wire, fp32 PSUM accumulation.
"""

import numpy as np
import ml_dtypes

import concourse.bass as bass
import concourse.mybir as mybir
import concourse.tile as tile
from concourse import bacc
from concourse.bass_utils import run_bass_kernel_spmd

N = 8192
D = 512
R = 20
TWO_R = 2 * R
L = 2
OUT = 256
P = 128
NCORES = 8
RPC = TWO_R // NCORES    # 5 relations per core
BLK = N // NCORES        # 1024 tokens per core
NSUB = BLK // P          # 8 sub-blocks of 128 tokens
HS = [4, 4]              # sub-blocks per half
B0 = HS[0] * P           # tokens per core in half 0
KC = D // P              # 4 contraction chunks
CH_IDX = 4 * P           # idxs per transposing-gather chunk

BF16 = ml_dtypes.bfloat16

LAST_EXEC_TIME_NS = None
LAST_RESULTS = None

_CACHE = {}


def _pack_idx16(idx: np.ndarray) -> np.ndarray:
    Ln = len(idx)
    assert Ln % 16 == 0
    base = idx.astype(np.int16).reshape(Ln // 16, 16).T
    return np.tile(base, (8, 1)).copy()


def _plan(dep_idx: np.ndarray, rel_idx: np.ndarray, gov_idx: np.ndarray):
    dep = dep_idx.astype(np.int64)
    gov = gov_idx.astype(np.int64)
    rel = rel_idx.astype(np.int64)

    dest = np.concatenate([dep, gov])
    src = np.concatenate([gov, dep])
    r2 = np.concatenate([rel, rel + R])

    owner = r2 // RPC
    slot = r2 % RPC
    peer = dest // BLK
    sub = (dest % BLK) // P
    half = (sub >= HS[0]).astype(np.int64)
    ksub = sub - HS[0] * half          # sub index within the half
    # single-AllGather layout: no source-half split needed
    shalf = np.zeros(2 * N, dtype=np.int64)

    # ---- per-half wire layout: [peer][ksub R1M rows ++ OVER] ----
    cnt = np.zeros((NCORES, NCORES, NSUB), dtype=np.int64)
    np.add.at(cnt, (owner, peer, sub), 1)

    R1M = [0, 0]
    OVER = [0, 0]
    SEG = [0, 0]
    for h in range(2):
        klo = 0 if h == 0 else HS[0]
        khi = klo + HS[h]

        def over_for(r1m):
            ov = 0
            for c in range(NCORES):
                for p in range(NCORES):
                    tot = sum(
                        max(0, int(cnt[c, p, kk]) - r1m)
                        for kk in range(klo, khi)
                    )
                    ov = max(ov, tot)
            return int(np.ceil(ov / 16) * 16) if ov else 0

        best = None
        for r1m in (16, 32, 48, 64):
            ov = over_for(r1m)
            seg = HS[h] * r1m + ov
            chunks = (NCORES * r1m) // P + (NCORES * ov) // P
            key = (seg, chunks)
            if best is None or key < best[0]:
                best = (key, r1m, ov, seg)
        _, r1m, ov, seg = best
        if ov == 0:
            ov = 16
            seg = HS[h] * r1m + ov
        R1M[h], OVER[h], SEG[h] = r1m, ov, seg
    SENDH = [NCORES * SEG[0], NCORES * SEG[1]]
    J2M = [NCORES * R1M[h] // P for h in range(2)]
    J2O = [NCORES * OVER[h] // P for h in range(2)]
    JT = [J2M[h] + J2O[h] for h in range(2)]
    koff = []
    o = 0
    for k in range(NSUB):
        koff.append(o)
        o += JT[0 if k < HS[0] else 1]
    NCHUNK = o

    # ---- GEMM tiles per (half, slot, src-part): src0 tiles are pure by
    # construction (messages bucketed by source half), and ordered first ----
    tps = np.zeros((2, RPC, 2), dtype=np.int64)
    for h in range(2):
        for s in range(RPC):
            for q in range(2):
                mx = 1 if q == 0 else 0
                for c in range(NCORES):
                    n = int(((owner == c) & (half == h) & (slot == s)
                             & (shalf == q)).sum())
                    mx = max(mx, (n + P - 1) // P)
                tps[h, s, q] = mx
    MTH = [int(tps[0].sum()), int(tps[1].sum())]
    MT = MTH[0] + MTH[1]
    NMSG = MT * P

    tile_slot = []        # per global tile: its relation slot
    tile_of = {}          # (h, s, q, t) -> global tile index
    halves = []
    g = 0
    for h in range(2):
        t_lo = g
        npure = int(tps[h, :, 0].sum())
        for q in range(2):
            for s in range(RPC):
                for t in range(int(tps[h, s, q])):
                    tile_of[(h, s, q, t)] = g
                    tile_slot.append(s)
                    g += 1
        halves.append(dict(
            tile_lo=t_lo, tile_hi=g, row_lo=t_lo * P, row_hi=g * P,
            npure_tiles=npure,
        ))

    # ---- per-message row assignment + wire slots ----
    msg_row = np.zeros(2 * N, dtype=np.int64)
    send_slot = np.zeros(2 * N, dtype=np.int64)
    of_pos = np.zeros(2 * N, dtype=np.int64) - 1
    for c in range(NCORES):
        rfill = np.zeros((NCORES, NSUB), dtype=np.int64)
        ofill = np.zeros((NCORES, 2), dtype=np.int64)
        for h in range(2):
            for s in range(RPC):
                for q in range(2):
                    m = np.nonzero((owner == c) & (half == h) & (slot == s)
                                   & (shalf == q))[0]
                    for pos, mm in enumerate(m):
                        t = pos // P
                        msg_row[mm] = tile_of[(h, s, q, t)] * P + pos % P
                        p = peer[mm]
                        rpos = rfill[p, sub[mm]]
                        rfill[p, sub[mm]] += 1
                        base = p * SEG[h]
                        if rpos < R1M[h]:
                            send_slot[mm] = base + ksub[mm] * R1M[h] + rpos
                        else:
                            op_ = ofill[p, h]
                            assert op_ < OVER[h]
                            ofill[p, h] += 1
                            of_pos[mm] = op_
                            send_slot[mm] = base + HS[h] * R1M[h] + op_

    cores = []
    for c in range(NCORES):
        cm = np.nonzero(owner == c)[0]
        idxA = np.zeros(NMSG, dtype=np.int64)
        idxA[msg_row[cm]] = src[cm]

        # scatter slots in GEMM-row order (within-half); pads -> trash rows
        idxS = np.zeros(NMSG, dtype=np.int64)
        for h in range(2):
            lo, hi = halves[h]["row_lo"], halves[h]["row_hi"]
            idxS[lo:hi] = SENDH[h] + np.arange(hi - lo)      # default: trash
        idxS[msg_row[cm]] = send_slot[cm]

        # one-hot matrices against the strided recv-load layout
        S = np.zeros((NCHUNK, P, P), dtype=np.float32)
        dm = np.nonzero(peer == c)[0]
        for m in dm:
            k = sub[m]
            h = 0 if k < HS[0] else 1
            dloc = (dest[m] - c * BLK) % P
            if of_pos[m] < 0:
                pos = send_slot[m] - c * SEG[h] - ksub[m] * R1M[h]
                rr = owner[m] * R1M[h] + pos
                S[koff[k] + rr % J2M[h], rr // J2M[h], dloc] = 1.0
            else:
                rr2 = owner[m] * OVER[h] + of_pos[m]
                S[koff[k] + J2M[h] + rr2 % J2O[h], rr2 // J2O[h], dloc] = 1.0

        CT = np.zeros((1 + TWO_R, BLK), dtype=np.float32)
        CT[0, :] = 1.0
        for m in dm:
            CT[1 + r2[m], dest[m] - c * BLK] += 1.0

        cores.append(
            dict(
                idxA=_pack_idx16(idxA),
                idxS=_pack_idx16(idxS),
                S=S.reshape(NCHUNK * P, P).astype(BF16),
                CT=CT.astype(BF16),
            )
        )

    return dict(
        MT=MT, MTH=MTH, tile_slot=tile_slot, NMSG=NMSG, R1M=R1M, OVER=OVER,
        SEG=SEG, SENDH=SENDH, J2M=J2M, J2O=J2O, JT=JT, koff=koff,
        NCHUNK=NCHUNK, halves=halves, cores=cores,
    )


def _build(plan):
    MTH = plan["MTH"]
    tile_slot = plan["tile_slot"]
    NMSG = plan["NMSG"]
    R1M, OVER, SEG, SENDH = plan["R1M"], plan["OVER"], plan["SEG"], plan["SENDH"]
    J2M, J2O, koff = plan["J2M"], plan["J2O"], plan["koff"]
    NCHUNK = plan["NCHUNK"]
    halves = plan["halves"]

    nc = bacc.Bacc(
        "TRN2",
        target_bir_lowering=False,
        debug=False,
        enable_asserts=True,
        num_devices=NCORES,
        num_swdge_queues=4,
    )
    dt = mybir.dt

    x0 = nc.dram_tensor("x0", [N, D], dt.bfloat16, kind="ExternalInput")
    x_own = nc.dram_tensor("x_own", [BLK, D], dt.bfloat16, kind="ExternalInput")
    wrel = nc.dram_tensor("wrel", [L, RPC, D, D], dt.bfloat16, kind="ExternalInput")
    wselfT = nc.dram_tensor("wselfT", [L, D, D], dt.bfloat16, kind="ExternalInput")
    bias = nc.dram_tensor("bias", [L, 1 + TWO_R, D], dt.bfloat16, kind="ExternalInput")
    ct = nc.dram_tensor("ct", [1 + TWO_R, BLK], dt.bfloat16, kind="ExternalInput")
    wffT = nc.dram_tensor("wffT", [D, OUT], dt.bfloat16, kind="ExternalInput")
    bff = nc.dram_tensor("bff", [1, OUT], dt.bfloat16, kind="ExternalInput")
    ident = nc.dram_tensor("ident", [P, P], dt.bfloat16, kind="ExternalInput")
    idxA = nc.dram_tensor("idxA", [P, NMSG // 16], dt.int16, kind="ExternalInput")
    idxS = nc.dram_tensor("idxS", [P, NMSG // 16], dt.int16, kind="ExternalInput")
    s_in = nc.dram_tensor("s", [NCHUNK * P, P], dt.bfloat16, kind="ExternalInput")
    y = nc.dram_tensor("y", [BLK, OUT], dt.float32, kind="ExternalOutput")

    h_own = nc.dram_tensor("h_own", [BLK, D], dt.bfloat16)
    h_full = nc.dram_tensor("h_full", [N, D], dt.bfloat16, addr_space="Shared")
    send = [
        [
            nc.dram_tensor(f"send{ll}_{h}", [SENDH[h] + MTH[h] * P, D], dt.bfloat16)
            for h in range(2)
        ]
        for ll in range(L)
    ]
    recv = [
        [nc.dram_tensor(f"recv{ll}_{h}", [SENDH[h], D], dt.bfloat16) for h in range(2)]
        for ll in range(L)
    ]

    Relu = mybir.ActivationFunctionType.Relu
    RG = [list(range(NCORES))]

    Q_G0 = 0      # layer-0 gathers + mainline bulk loads
    Q_S0 = 1      # layer-0 scatters
    Q_G1 = 2      # layer-1 gathers (alternating with q0)
    Q_S1 = 3      # layer-1 scatters

    def half_chunks(h):
        lo, hi = halves[h]["row_lo"], halves[h]["row_hi"]
        pure_hi = lo + halves[h]["npure_tiles"] * P
        out = []
        ci = 0
        for clo in range(lo, hi, CH_IDX):
            chi = min(clo + CH_IDX, hi)
            out.append((ci, clo, chi, chi <= pure_hi))
            ci += 1
        return out

    with tile.TileContext(nc) as tc:
        with (
            tc.tile_pool(name="const", bufs=1) as const,
            tc.tile_pool(name="xtc", bufs=1) as xtcp,
            tc.tile_pool(name="xself", bufs=1) as xsp,
            tc.tile_pool(name="mso", bufs=1) as msop,
            tc.tile_pool(name="msgb", bufs=4) as msgbp,
            tc.tile_pool(name="selfb", bufs=8) as selfbp,
            tc.tile_pool(name="selfb1", bufs=8) as selfb1p,
            tc.tile_pool(name="hT", bufs=8) as hTp,
            tc.tile_pool(name="h", bufs=3) as hp,
            tc.tile_pool(name="psum_m", bufs=3, space="PSUM") as psum_m,
            tc.tile_pool(name="psum_o", bufs=2, space="PSUM") as psum_o,
            tc.tile_pool(name="psum_y", bufs=1, space="PSUM") as psum_y,
            tc.tile_pool(name="psum_tr", bufs=1, space="PSUM") as psum_tr,
        ):
            # ---- small latency-critical loads on the HWDGE engine ----
            idxA_sb = const.tile([P, NMSG // 16], dt.int16)
            nc.scalar.dma_start(idxA_sb[:], idxA.ap())
            idxS_sb = const.tile([P, NMSG // 16], dt.int16)
            nc.scalar.dma_start(idxS_sb[:], idxS.ap())
            ident_sb = const.tile([P, P], dt.bfloat16)
            nc.scalar.dma_start(ident_sb[:], ident.ap())
            ct_sb = const.tile([1 + TWO_R, BLK], dt.bfloat16)
            nc.sync.dma_start(ct_sb[:], ct.ap())
            bias_sb = const.tile([1 + TWO_R, L, D], dt.bfloat16)
            nc.sync.dma_start(bias_sb[:], bias.ap().rearrange("l b d -> b l d"))
            x_row = xsp.tile([P, NSUB, D], dt.bfloat16, tag="xrow")
            nc.sync.dma_start(
                x_row[:], x_own.ap().rearrange("(k p) d -> p k d", p=P)
            )
            wselfT_sb = const.tile([P, L, KC, D], dt.bfloat16)
            nc.sync.dma_start(
                wselfT_sb[:], wselfT.ap().rearrange("l (c p) n -> p l c n", p=P)
            )

            zero_sb = const.tile([P, 4, D], dt.bfloat16)
            nc.vector.memset(zero_sb[:], 0.0)
            ones_sb = const.tile([1, P], dt.bfloat16)
            nc.vector.memset(ones_sb[:], 1.0)

            # warm-up A2A (256KB): pays the ncfw cold-start during startup;
            # the first real A2A is not ready before ~90us so this is free.
            warm_in = nc.dram_tensor("warm_in", [P, 1024], dt.bfloat16)
            warm_out = nc.dram_tensor("warm_out", [P, 1024], dt.bfloat16)
            nc.sync.dma_start(
                warm_in.ap().rearrange("p (c d) -> p c d", c=2), zero_sb[:, 0:2, :]
            )
            nc.gpsimd.collective_compute(
                "AllToAll",
                mybir.AluOpType.bypass,
                replica_groups=RG,
                ins=[warm_in.ap()],
                outs=[warm_out.ap()],
            )

            def zero_wire(ll, h):
                zrows = P * 4
                for lo in range(0, SENDH[h], zrows):
                    hi = min(lo + zrows, SENDH[h])
                    nc.gpsimd.dma_start(
                        send[ll][h].ap()[lo:hi, :],
                        zero_sb[:, : (hi - lo) // P, :],
                    )

            def make_xc(h, ci, nrow):
                xc = xtcp.tile(
                    [P, KC, nrow], dt.bfloat16, tag=f"xc{h}_{ci}", bufs=1,
                    name=f"xc{h}_{ci}",
                )
                return xc

            wrel_sb = [[None] * RPC for _ in range(L)]
            xc0 = {}
            for h in range(2):
                for ci, clo, chi, _pure in half_chunks(h):
                    xc = make_xc(h, ci, chi - clo)
                    nc.gpsimd.dma_gather(
                        out_ap=xc[:],
                        in_ap=x0.ap(),
                        idxs_ap=idxA_sb[:, clo // 16 : chi // 16],
                        num_idxs=chi - clo,
                        num_idxs_reg=chi - clo,
                        elem_size=D,
                        transpose=True,
                        queue_num=Q_G0,
                    )
                    xc0[(h, ci)] = xc
                if h == 0:
                    zero_wire(0, 0)
                    for ss in range(RPC):
                        wt = const.tile(
                            [P, KC, D], dt.bfloat16, tag=f"wrel0_{ss}",
                            name=f"wrel0_{ss}",
                        )
                        nc.gpsimd.dma_start(
                            wt[:],
                            wrel.ap()[0, ss].rearrange("(c p) n -> p c n", p=P),
                        )
                        wrel_sb[0][ss] = wt
            zero_wire(0, 1)
            s_sb = const.tile([P, NCHUNK, P], dt.bfloat16)
            nc.gpsimd.dma_start(
                s_sb[:], s_in.ap().rearrange("(c p) n -> p c n", p=P)
            )

            # PE-transpose x_own into xself0 (lhsT layout for the self GEMMs)
            xself0 = xsp.tile([P, KC, BLK], dt.bfloat16, tag="xself")
            for k in range(NSUB):
                ptr = psum_tr.tile([P, KC, P], dt.bfloat16, space="PSUM", tag="ptr")
                for kc in range(KC):
                    nc.tensor.transpose(
                        ptr[:, kc, :], x_row[:, k, kc * P : (kc + 1) * P],
                        ident_sb[:],
                    )
                nc.vector.tensor_copy(xself0[:, :, k * P : (k + 1) * P], ptr[:])

            mso = [
                msop.tile(
                    [P, MTH[h], D], dt.bfloat16, tag=f"mso{h}", bufs=1,
                    name=f"mso{h}",
                )
                for h in range(2)
            ]

            def scatter_batch(layer, h, q, ta, tb):
                # scatter GEMM tiles [ta, tb) of half h right behind their
                # copies, so the A2A doorbell trails the last GEMM by only
                # one small scatter batch
                t0 = halves[h]["tile_lo"]
                lo, hi = (t0 + ta) * P, (t0 + tb) * P
                nc.gpsimd.dma_scatter_add(
                    send[layer][h].ap(),
                    mso[h][:, ta:tb, :],
                    idxS_sb[:, lo // 16 : hi // 16],
                    hi - lo,
                    hi - lo,
                    D,
                    queue_num=q,
                )

            selfb0 = [None] * NSUB
            selfb1 = [None] * NSUB

            def selfb0_compute(ks):
                for k in ks:
                    pm = psum_m.tile([P, D], dt.float32, space="PSUM", tag="pmsg")
                    for kc in range(KC):
                        nc.tensor.matmul(
                            out=pm[:],
                            lhsT=xself0[:, kc, k * P : (k + 1) * P],
                            rhs=wselfT_sb[:, 0, kc, :],
                            start=(kc == 0),
                            stop=False,
                        )
                    nc.tensor.matmul(
                        out=pm[:],
                        lhsT=ct_sb[:, k * P : (k + 1) * P],
                        rhs=bias_sb[:, 0, :],
                        start=False,
                        stop=True,
                    )
                    sb = selfbp.tile([P, D], dt.float32, tag="selfb")
                    nc.vector.tensor_copy(sb[:], pm[:])
                    selfb0[k] = sb

            def msg_gemms(layer, h, xcs, q):
                t0, t1 = halves[h]["tile_lo"], halves[h]["tile_hi"]
                nt = t1 - t0
                sc_lo = 0
                for mt in range(t0, t1):
                    tih = mt - t0
                    ci, off = (tih * P) // CH_IDX, (tih * P) % CH_IDX
                    xc = xcs[(h, ci)]
                    ss = tile_slot[mt]
                    pm = psum_m.tile([P, D], dt.float32, space="PSUM", tag="pmsg")
                    for kc in range(KC):
                        nc.tensor.matmul(
                            out=pm[:],
                            lhsT=xc[:, kc, off : off + P],
                            rhs=wrel_sb[layer][ss][:, kc, :],
                            start=(kc == 0),
                            stop=(kc == KC - 1),
                        )
                    nc.vector.tensor_copy(mso[h][:, tih, :], pm[:])
                    if tih + 1 - sc_lo >= 4 and nt - (tih + 1) >= 2:
                        scatter_batch(layer, h, q, sc_lo, tih + 1)
                        sc_lo = tih + 1
                if sc_lo < nt:
                    scatter_batch(layer, h, q, sc_lo, nt)

            def a2a(layer, h):
                nc.gpsimd.collective_compute(
                    "AllToAll",
                    mybir.AluOpType.bypass,
                    replica_groups=RG,
                    ins=[send[layer][h].ap()[: SENDH[h], :]],
                    outs=[recv[layer][h].ap()],
                )

            # ================= layer 0 message phase =================
            selfb0_compute(range(NSUB))
            msg_gemms(0, 0, xc0, Q_S0)
            a2a(0, 0)
            msg_gemms(0, 1, xc0, Q_S0)
            a2a(0, 1)

            # layer-1-only consts + wire zeroing (SWDGE mainline).  wrel
            # layer-1 reuses the layer-0 buffers (WAR: waits the l0 GEMMs).
            for ss in range(RPC):
                wt = const.tile(
                    [P, KC, D], dt.bfloat16, tag=f"wrel0_{ss}", name=f"wrel1_{ss}"
                )
                nc.gpsimd.dma_start(
                    wt[:], wrel.ap()[1, ss].rearrange("(c p) n -> p c n", p=P)
                )
                wrel_sb[1][ss] = wt
            wffT_sb = const.tile([P, KC, OUT], dt.bfloat16)
            nc.gpsimd.dma_start(
                wffT_sb[:], wffT.ap().rearrange("(c p) n -> p c n", p=P)
            )
            bff_sb = const.tile([1, OUT], dt.bfloat16)
            nc.gpsimd.dma_start(bff_sb[:], bff.ap())
            zero_wire(1, 0)
            zero_wire(1, 1)

            def load_recv(layer, hh):
                seg = recv[layer][hh].ap().rearrange("(s g) d -> s g d", s=NCORES)
                ov = msgbp.tile(
                    [P, J2O[hh], D], dt.bfloat16, tag=f"msgO{hh}", bufs=2,
                    name=f"ov{hh}",
                )
                nc.scalar.dma_start(
                    ov[:],
                    seg[:, HS[hh] * R1M[hh] : HS[hh] * R1M[hh] + OVER[hh], :],
                )
                mbs = []
                for kl in range(HS[hh]):
                    mb = msgbp.tile(
                        [P, J2M[hh], D], dt.bfloat16, tag=f"msgB{hh}_{kl % 3}",
                        bufs=2, name=f"mb{hh}_{kl}",
                    )
                    nc.scalar.dma_start(
                        mb[:], seg[:, kl * R1M[hh] : (kl + 1) * R1M[hh], :]
                    )
                    mbs.append(mb)
                return ov, mbs

            def accum_k(layer, hh, k, mb, ov):
                po = psum_o.tile([P, D], dt.float32, space="PSUM", tag="pout")
                for j in range(J2M[hh]):
                    nc.tensor.matmul(
                        out=po[:],
                        lhsT=s_sb[:, koff[k] + j, :],
                        rhs=mb[:, j, :],
                        start=(j == 0),
                        stop=False,
                    )
                for j in range(J2O[hh]):
                    nc.tensor.matmul(
                        out=po[:],
                        lhsT=s_sb[:, koff[k] + J2M[hh] + j, :],
                        rhs=ov[:, j, :],
                        start=False,
                        stop=(j == J2O[hh] - 1),
                    )
                sb = selfb0[k] if layer == 0 else selfb1[k]
                nc.vector.tensor_add(out=po[:], in0=po[:], in1=sb[:])
                return po

            hT = [None] * NSUB

            def accum_half_l0(hh):
                ov, mbs = load_recv(0, hh)
                for kl in range(HS[hh]):
                    k = (0 if hh == 0 else HS[0]) + kl
                    po = accum_k(0, hh, k, mbs[kl], ov)
                    hsb = hp.tile([P, D], dt.bfloat16, tag="hsb")
                    nc.scalar.activation(hsb[:], po[:], Relu)
                    nc.scalar.dma_start(
                        h_own.ap()[k * P : (k + 1) * P, :], hsb[:]
                    )
                    # h^T (PE transpose) for the direct layer-1 self GEMM
                    ptr = psum_tr.tile([P, KC, P], dt.bfloat16, space="PSUM", tag="ptr")
                    for kc in range(KC):
                        nc.tensor.transpose(
                            ptr[:, kc, :], hsb[:, kc * P : (kc + 1) * P], ident_sb[:]
                        )
                    ht = hTp.tile([P, KC, P], dt.bfloat16, tag="hT")
                    nc.vector.tensor_copy(ht[:], ptr[:])
                    hT[k] = ht

            def selfb1_compute():
                # layer-1 self+bias tiles; runs under the AllGather (the AG
                # trigger only waits on the h stores, not on these GEMMs)
                for k in range(NSUB):
                    ps = psum_m.tile([P, D], dt.float32, space="PSUM", tag="pmsg")
                    for cc in range(KC):
                        nc.tensor.matmul(
                            out=ps[:],
                            lhsT=hT[k][:, cc, :],
                            rhs=wselfT_sb[:, 1, cc, :],
                            start=(cc == 0),
                            stop=False,
                        )
                    nc.tensor.matmul(
                        out=ps[:],
                        lhsT=ct_sb[:, k * P : (k + 1) * P],
                        rhs=bias_sb[:, 1, :],
                        start=False,
                        stop=True,
                    )
                    sb1 = selfb1p.tile([P, D], dt.float32, tag="selfb1")
                    nc.vector.tensor_copy(sb1[:], ps[:])
                    selfb1[k] = sb1

            def accum_half_l1(hh):
                ov, mbs = load_recv(1, hh)
                for kl in range(HS[hh]):
                    k = (0 if hh == 0 else HS[0]) + kl
                    po = accum_k(1, hh, k, mbs[kl], ov)
                    h2 = hp.tile([P, D], dt.bfloat16, tag="h2")
                    nc.scalar.activation(h2[:], po[:], Relu)
                    ptr = psum_tr.tile([P, KC, P], dt.bfloat16, space="PSUM", tag="ptr")
                    for kc in range(KC):
                        nc.tensor.transpose(
                            ptr[:, kc, :], h2[:, kc * P : (kc + 1) * P], ident_sb[:]
                        )
                    h2t = hp.tile([P, KC, P], dt.bfloat16, tag="h2t")
                    nc.vector.tensor_copy(h2t[:], ptr[:])
                    # fused FF GEMM off h2^T
                    py_ = psum_y.tile([P, OUT], dt.float32, space="PSUM", tag="py")
                    for kc in range(KC):
                        nc.tensor.matmul(
                            out=py_[:],
                            lhsT=h2t[:, kc, :],
                            rhs=wffT_sb[:, kc, :],
                            start=(kc == 0),
                            stop=False,
                        )
                    nc.tensor.matmul(
                        out=py_[:], lhsT=ones_sb[:], rhs=bff_sb[:],
                        start=False, stop=True,
                    )
                    ysb = hp.tile([P, OUT], dt.float32, tag="ysb")
                    nc.vector.tensor_copy(ysb[:], py_[:])
                    nc.scalar.dma_start(y.ap()[k * P : (k + 1) * P, :], ysb[:])

            # ================= layer 0 accumulation =================
            accum_half_l0(0)
            accum_half_l0(1)
            # single AllGather of h (core-major h_full layout)
            nc.gpsimd.collective_compute(
                "AllGather",
                mybir.AluOpType.bypass,
                replica_groups=RG,
                ins=[h_own.ap()],
                outs=[h_full.ap()],
            )
            selfb1_compute()

            # ================= layer 1 =================
            # gathers from h_full; src0-pure chunks read only the AG-half-0
            # region, so they fire while AG-half-1 is still on the wire
            xc1 = {}
            qi = 0
            for h in range(2):
                for ci, clo, chi, pure in half_chunks(h):
                    xc = make_xc(h, ci, chi - clo)
                    nc.gpsimd.dma_gather(
                        out_ap=xc[:],
                        in_ap=h_full.ap(),
                        idxs_ap=idxA_sb[:, clo // 16 : chi // 16],
                        num_idxs=chi - clo,
                        num_idxs_reg=chi - clo,
                        elem_size=D,
                        transpose=True,
                        queue_num=(Q_G1 if qi % 2 == 0 else Q_G0),
                    )
                    qi += 1
                    xc1[(h, ci)] = xc
            msg_gemms(1, 0, xc1, Q_S1)
            a2a(1, 0)
            msg_gemms(1, 1, xc1, Q_S1)
            a2a(1, 1)

            accum_half_l1(0)
            accum_half_l1(1)

    nc.compile()
    return nc


def _in_maps(plan, x, W_self, b_self, W_rel, b_rel, W_ff, b_ff):
    x0 = x.astype(BF16)
    wselfT = np.ascontiguousarray(W_self.transpose(0, 2, 1)).astype(BF16)
    bias = np.concatenate([b_self[:, None, :], b_rel], axis=1).astype(BF16)
    wffT = np.ascontiguousarray(W_ff.T).astype(BF16)
    bffr = b_ff.reshape(1, OUT).astype(BF16)
    wrelT_all = np.ascontiguousarray(W_rel.transpose(0, 1, 3, 2)).astype(BF16)
    identm = np.eye(P, dtype=BF16)

    in_maps = []
    for c in range(NCORES):
        t = plan["cores"][c]
        in_maps.append(
            {
                "x0": x0,
                "x_own": np.ascontiguousarray(x0[c * BLK : (c + 1) * BLK]),
                "wrel": np.ascontiguousarray(wrelT_all[:, c * RPC : (c + 1) * RPC]),
                "wselfT": wselfT,
                "bias": bias,
                "ct": t["CT"],
                "wffT": wffT,
                "bff": bffr,
                "ident": identm,
                "idxA": t["idxA"],
                "idxS": t["idxS"],
                "s": t["S"],
            }
        )
    return in_maps


def kernel(x, dep_idx, rel_idx, gov_idx, W_self, b_self, W_rel, b_rel, W_ff, b_ff):
    global LAST_EXEC_TIME_NS, LAST_RESULTS

    x = np.asarray(x)
    dep_idx = np.asarray(dep_idx)
    rel_idx = np.asarray(rel_idx)
    gov_idx = np.asarray(gov_idx)
    W_self = np.asarray(W_self)
    b_self = np.asarray(b_self)
    W_rel = np.asarray(W_rel)
    b_rel = np.asarray(b_rel)
    W_ff = np.asarray(W_ff)
    b_ff = np.asarray(b_ff)
    assert x.shape == (N, D) and W_rel.shape == (L, TWO_R, D, D)

    key = (dep_idx.tobytes(), rel_idx.tobytes(), gov_idx.tobytes())
    if key in _CACHE:
        nc, plan = _CACHE[key]
    else:
        plan = _plan(dep_idx, rel_idx, gov_idx)
        nc = _build(plan)
        _CACHE.clear()
        _CACHE[key] = (nc, plan)

    in_maps = _in_maps(plan, x, W_self, b_self, W_rel, b_rel, W_ff, b_ff)
    res = run_bass_kernel_spmd(nc, in_maps, list(range(NCORES)))
    LAST_EXEC_TIME_NS = res.exec_time_ns
    LAST_RESULTS = res
    out = np.concatenate([res.results[c]["y"] for c in range(NCORES)], axis=0)
    return out.astype(np.float32)
